# revision 8
# baseline (speedup 1.0000x reference)
"""CopyMechanism (pointer-generator) kernel for 8 Trainium2 NeuronCores.

Full problem: B=16, T=128, H=512, V=32000, S=400.
  gen = sigmoid(ctx@wh + hid@ws + trg@wx + b)          [B,T,1]
  out = gen * vocab_dists; out[b,t,ids[b,t,s]] += (1-gen)*attn[b,t,s]

Sharding: data-parallel over batch. Core i handles batches [2i, 2i+1]
(256 rows of T-steps). Weights replicated. No cross-core communication.

Device algorithm (per core, per row r):
  Decompose vocab index v = p*250 + f  (V = 128*250), so a row's 32000-wide
  output is an SBUF tile [128 partitions, 250 free].  The scatter-add of the
  S=400 attn values becomes a sum of outer products:
     M[p,f] = sum_s onehot(pi[s])[p] * (onehot(fi[s])[f] * val[s])
  computed by TensorE matmuls contracting s (4 chunks of <=128 on the
  partition axis).  One-hots are built on VectorE with iota/is_equal
  tensor_scalar ops (indices pre-decomposed and pre-transposed on host --
  integer-only preprocessing).  The base p_gen*vocab is pre-written into
  PSUM by ScalarE (activation Copy with per-row scale), matmuls accumulate
  on top (start=False), ScalarE copies PSUM->SBUF, DMA out.

  p_gen is computed on-device (dot products + sigmoid), bounced through a
  DRAM scratch and re-loaded with a partition-broadcast AP so each row's
  scalar is available on all 128 partitions.
"""

import numpy as np

# ---------------------------------------------------------------------------
# problem constants (hardcoded per contract)
B, T, H, V, S = 16, 128, 512, 32000, 400
N_CORES = 8
BPC = B // N_CORES          # batches per core
R_FULL = BPC * T            # rows per core = 256
FD_FULL = V // 128          # 250
SP_FULL = (S + 127) // 128  # 4 s-chunks
G_FULL = 16                 # rows per vocab DMA group

_PROGRAM_CACHE = {}


def build_program(R=R_FULL, FD=FD_FULL, SP=SP_FULL, G=G_FULL, mode="diag"):
    """Build + compile the per-core Bass program. Same program for all cores.

    R : rows per core (multiple of 128)
    FD: free-dim width of the vocab decomposition (V_local = 128*FD)
    SP: number of 128-wide s-chunks (S padded to SP*128)
    G : rows per vocab/out DMA group
    mode: "diag" -> base p_gen*vocab via a diagonal matmul starting the PSUM
          group; "dve" -> base+merge on VectorE after the scatter matmuls.
    """
    key = (R, FD, SP, G, mode)
    if key in _PROGRAM_CACHE:
        return _PROGRAM_CACHE[key]

    from contextlib import ExitStack

    import concourse.bass as bass
    import concourse.tile as tile
    from concourse import bacc, mybir

    f32 = mybir.dt.float32
    Alu = mybir.AluOpType
    Act = mybir.ActivationFunctionType
    VL = 128 * FD
    RB = R // 128
    NG = R // G
    assert R % 128 == 0 and R % G == 0

    nc = bacc.Bacc("TRN2", target_bir_lowering=False, debug=False)

    ctx_d = nc.dram_tensor("ctx", [R, H], f32, kind="ExternalInput")
    hid_d = nc.dram_tensor("hid", [R, H], f32, kind="ExternalInput")
    trg_d = nc.dram_tensor("trg", [R, H], f32, kind="ExternalInput")
    vocab_d = nc.dram_tensor("vocab", [R, VL], f32, kind="ExternalInput")
    attnT_d = nc.dram_tensor("attnT", [128, RB * SP, 128], f32, kind="ExternalInput")
    piT_d = nc.dram_tensor("piT", [128, RB * SP, 128], f32, kind="ExternalInput")
    fiT_d = nc.dram_tensor("fiT", [128, RB * SP, 128], f32, kind="ExternalInput")
    # weights replicated across partitions on host (pure data movement)
    wh_d = nc.dram_tensor("wh", [128, H], f32, kind="ExternalInput")
    ws_d = nc.dram_tensor("ws", [128, H], f32, kind="ExternalInput")
    wx_d = nc.dram_tensor("wx", [128, H], f32, kind="ExternalInput")
    wxb_d = nc.dram_tensor("wxb", [128, 1], f32, kind="ExternalInput")
    iotaP_d = nc.dram_tensor("iotaP", [128, 128], f32, kind="ExternalInput")
    iotaF_d = nc.dram_tensor("iotaF", [128, FD], f32, kind="ExternalInput")
    ident_d = nc.dram_tensor("ident", [128, 128], f32, kind="ExternalInput")
    out_d = nc.dram_tensor("out", [R, VL], f32, kind="ExternalOutput")

    with tile.TileContext(nc) as tc, ExitStack() as es:
        singles = es.enter_context(tc.tile_pool(name="singles", bufs=1))
        ph1 = es.enter_context(tc.tile_pool(name="ph1", bufs=2))
        vpool = es.enter_context(tc.tile_pool(name="vpool", bufs=2))
        opool = es.enter_context(tc.tile_pool(name="opool", bufs=2))
        abpool = es.enter_context(tc.tile_pool(name="ab", bufs=4))
        ppool = es.enter_context(tc.tile_pool(name="psum", bufs=6, space="PSUM"))
        dpool = es.enter_context(tc.tile_pool(name="dram", bufs=1, space="DRAM"))

        # --- constants / small inputs ---
        attnT = singles.tile([128, RB * SP, 128], f32)
        nc.sync.dma_start(attnT[:], attnT_d[:])
        piT = singles.tile([128, RB * SP, 128], f32)
        nc.sync.dma_start(piT[:], piT_d[:])
        fiT = singles.tile([128, RB * SP, 128], f32)
        nc.sync.dma_start(fiT[:], fiT_d[:])
        iotaP = singles.tile([128, 128], f32)
        nc.sync.dma_start(iotaP[:], iotaP_d[:])
        iotaF = singles.tile([128, FD], f32)
        nc.sync.dma_start(iotaF[:], iotaF_d[:])
        ident = singles.tile([128, 128], f32)
        nc.sync.dma_start(ident[:], ident_d[:])
        wh = singles.tile([128, H], f32)
        nc.sync.dma_start(wh[:], wh_d[:])
        ws = singles.tile([128, H], f32)
        nc.sync.dma_start(ws[:], ws_d[:])
        wx = singles.tile([128, H], f32)
        nc.sync.dma_start(wx[:], wx_d[:])
        wxb = singles.tile([128, 1], f32)
        nc.sync.dma_start(wxb[:], wxb_d[:])
        scaledT = singles.tile([128, RB * SP, 128], f32)
        pgen_all = singles.tile([128, R], f32)
        om_all = singles.tile([128, R], f32)
        pgen_dram = dpool.tile([R, 1], f32)

        # --- phase 1a: p_gen per row (rows on partitions), bounce to DRAM ---
        for blk in range(RB):
            rows = slice(blk * 128, (blk + 1) * 128)
            gacc = ph1.tile([128, 1], f32, tag="gacc")
            gtmp = ph1.tile([128, 1], f32, tag="gtmp")
            g2 = ph1.tile([128, 1], f32, tag="g2")
            prod = ph1.tile([128, H], f32, tag="prod")
            for i, (src_d, w) in enumerate(
                ((ctx_d, wh), (hid_d, ws), (trg_d, wx))
            ):
                x = ph1.tile([128, H], f32, tag="x")
                nc.sync.dma_start(x[:], src_d[rows, :])
                nc.vector.tensor_tensor(prod[:], x[:], w[:], op=Alu.mult)
                dst = (gacc, gtmp, g2)[i]
                nc.vector.tensor_reduce(
                    dst[:], prod[:], axis=mybir.AxisListType.X, op=Alu.add
                )
            gsum = ph1.tile([128, 1], f32, tag="gsum")
            nc.vector.tensor_tensor(gsum[:], gacc[:], gtmp[:], op=Alu.add)
            gall = ph1.tile([128, 1], f32, tag="gall")
            nc.vector.tensor_tensor(gall[:], gsum[:], g2[:], op=Alu.add)
            pgen_col = ph1.tile([128, 1], f32, tag="pgen")
            nc.scalar.activation(
                pgen_col[:], gall[:], Act.Sigmoid, bias=wxb[:], scale=1.0
            )
            nc.sync.dma_start(pgen_dram[rows, :], pgen_col[:])

        # --- phase 1b: broadcast p_gen to all partitions; scaled attnT ---
        pg_flat = pgen_dram[:, 0]
        pg_bcast = bass.AP(
            tensor=pg_flat.tensor, offset=pg_flat.offset,
            ap=[[0, 128]] + list(pg_flat.ap),
        )
        nc.gpsimd.dma_start(pgen_all[:], pg_bcast)
        nc.vector.tensor_scalar(
            om_all[:], pgen_all[:], -1.0, 1.0, Alu.mult, Alu.add
        )
        for blk in range(RB):
            for c in range(SP):
                nc.vector.tensor_tensor(
                    scaledT[:, blk * SP + c, :],
                    attnT[:, blk * SP + c, :],
                    om_all[:, blk * 128:(blk + 1) * 128],
                    op=Alu.mult,
                )

        # --- phase 2: per-row scatter-add via one-hot matmuls ---
        vocab_v = vocab_d[:].rearrange("r (p f) -> p r f", p=128)
        out_v = out_d[:].rearrange("r (p f) -> p r f", p=128)
        for grp in range(NG):
            gr = slice(grp * G, (grp + 1) * G)
            vt = vpool.tile([128, G, FD], f32)
            nc.sync.dma_start(vt[:], vocab_v[:, gr, :])
            ot = opool.tile([128, G, FD], f32)
            for j in range(G):
                r = grp * G + j
                blk = r // 128
                rl = r % 128
                ps = ppool.tile([128, FD], f32)
                pg_sc = pgen_all[:, r:r + 1]
                if mode == "diag":
                    D = abpool.tile([128, 128], f32, tag="D")
                    nc.vector.tensor_scalar(
                        D[:], ident[:], pg_sc, None, Alu.mult
                    )
                    nc.tensor.matmul(
                        ps[:], lhsT=D[:], rhs=vt[:, j, :],
                        start=True, stop=False,
                    )
                for c in range(SP):
                    ch = blk * SP + c
                    A = abpool.tile([128, 128], f32, tag="A")
                    nc.vector.tensor_scalar(
                        A[:], iotaP[:], piT[:, ch, rl:rl + 1], None, Alu.is_equal
                    )
                    Bt = abpool.tile([128, FD], f32, tag="B")
                    nc.vector.tensor_scalar(
                        Bt[:], iotaF[:], fiT[:, ch, rl:rl + 1],
                        scaledT[:, ch, rl:rl + 1], Alu.is_equal, Alu.mult,
                    )
                    nc.tensor.matmul(
                        ps[:], lhsT=A[:], rhs=Bt[:],
                        start=(False if mode == "diag" else c == 0),
                        stop=(c == SP - 1),
                    )
                if mode == "diag":
                    nc.scalar.copy(ot[:, j, :], ps[:])
                else:
                    nc.vector.tensor_scalar(
                        ot[:, j, :], vt[:, j, :], pg_sc, None, Alu.mult
                    )
                    nc.vector.tensor_tensor(
                        ot[:, j, :], ot[:, j, :], ps[:], op=Alu.add
                    )
            nc.sync.dma_start(out_v[:, gr, :], ot[:])

    nc.compile()
    _PROGRAM_CACHE[key] = nc
    return nc


def make_core_inputs(ctx, hid, trg, vocab, attn, ids, w_h, w_s, w_x_w, w_x_b,
                     R=R_FULL, FD=FD_FULL, SP=SP_FULL):
    """Host-side prep for one core: flatten rows, decompose + transpose indices.

    ctx/hid/trg: [R, H] f32; vocab: [R, 128*FD] f32; attn: [R, S'] f32;
    ids: [R, S'] int. Returns the in_map dict for this core.
    """
    RB = R // 128
    Sp = SP * 128
    Sl = attn.shape[1]
    f32 = np.float32

    ids = np.asarray(ids).astype(np.int64)
    pi = (ids // FD).astype(f32)
    fi = (ids % FD).astype(f32)

    def tr(x, pad):
        full = np.full((R, Sp), pad, dtype=f32)
        full[:, :Sl] = x
        # [R, Sp] -> [RB, 128(r), SP, 128(s)] -> [s, RB, SP, r]
        t = full.reshape(RB, 128, SP, 128).transpose(3, 0, 2, 1)
        return np.ascontiguousarray(t.reshape(128, RB * SP, 128))

    def rep(w, n):
        return np.ascontiguousarray(
            np.broadcast_to(np.asarray(w, dtype=f32).reshape(1, n), (128, n))
        )

    return {
        "ctx": np.ascontiguousarray(ctx, dtype=f32),
        "hid": np.ascontiguousarray(hid, dtype=f32),
        "trg": np.ascontiguousarray(trg, dtype=f32),
        "vocab": np.ascontiguousarray(vocab, dtype=f32),
        "attnT": tr(np.asarray(attn, dtype=f32), 0.0),
        "piT": tr(pi, 1.0e4),
        "fiT": tr(fi, -1.0),
        "wh": rep(w_h, H),
        "ws": rep(w_s, H),
        "wx": rep(w_x_w, H),
        "wxb": rep(w_x_b, 1),
        "iotaP": rep(np.arange(128, dtype=f32), 128),
        "iotaF": rep(np.arange(FD, dtype=f32), FD),
        "ident": np.eye(128, dtype=f32),
    }


def make_in_maps(context_vecs, hidden, trg_embs, vocab_dists, attn_dists,
                 src_ids, w_h, w_s, w_x_w, w_x_b):
    """Build the 8 per-core input dicts from full inputs."""
    context_vecs = np.asarray(context_vecs)
    hidden = np.asarray(hidden)
    trg_embs = np.asarray(trg_embs)
    vocab_dists = np.asarray(vocab_dists)
    attn_dists = np.asarray(attn_dists)
    src_ids = np.asarray(src_ids)

    in_maps = []
    for i in range(N_CORES):
        bs = slice(i * BPC, (i + 1) * BPC)
        in_maps.append(make_core_inputs(
            context_vecs[bs].reshape(R_FULL, H),
            hidden[bs].reshape(R_FULL, H),
            trg_embs[bs].reshape(R_FULL, H),
            vocab_dists[bs].reshape(R_FULL, V),
            attn_dists[bs].reshape(R_FULL, S),
            src_ids[bs].reshape(R_FULL, S),
            w_h, w_s, w_x_w, w_x_b,
        ))
    return in_maps


def kernel(context_vecs, hidden, trg_embs, vocab_dists, attn_dists,
           src_ids, pad_id, w_h, w_s, w_x_w, w_x_b):
    """Full-input entry point. Shards over 8 NeuronCores, returns [B,T,V] f32."""
    from concourse.bass_utils import run_bass_kernel_spmd

    nc = build_program()
    in_maps = make_in_maps(context_vecs, hidden, trg_embs, vocab_dists,
                           attn_dists, src_ids, w_h, w_s, w_x_w, w_x_b)
    res = run_bass_kernel_spmd(nc, in_maps, list(range(N_CORES)))
    outs = [np.asarray(res.results[i]["out"]).reshape(BPC, T, V)
            for i in range(N_CORES)]
    return np.concatenate(outs, axis=0)


# revision 13
# speedup vs baseline: 8.3236x; 8.3236x over previous
"""CopyMechanism (pointer-generator) kernel for 8 Trainium2 NeuronCores.

Full problem: B=16, T=128, H=512, V=32000, S=400.
  gen = sigmoid(ctx@wh + hid@ws + trg@wx + b)          [B,T,1]
  out = gen * vocab_dists; out[b,t,ids[b,t,s]] += (1-gen)*attn[b,t,s]

Sharding: data-parallel over batch. Core i handles batches [2i, 2i+1]
(256 rows of T-steps). Weights replicated. No cross-core communication.

Device algorithm (per core, per row r):
  Decompose vocab index v = p*250 + f  (V = 128*250), so a row's 32000-wide
  output is an SBUF tile [128 partitions, 250 free].  The scatter-add of the
  S=400 attn values becomes a sum of outer products:
     M[p,f] = sum_s onehot(pi[s])[p] * (onehot(fi[s])[f] * val[s])
  computed by TensorE matmuls contracting s (4 chunks of <=128 on the
  partition axis).  One-hots are built on VectorE with iota/is_equal
  tensor_scalar ops (indices pre-decomposed and pre-transposed on host --
  integer-only preprocessing).  The base p_gen*vocab is pre-written into
  PSUM by ScalarE (activation Copy with per-row scale), matmuls accumulate
  on top (start=False), ScalarE copies PSUM->SBUF, DMA out.

  p_gen is computed on-device (dot products + sigmoid), bounced through a
  DRAM scratch and re-loaded with a partition-broadcast AP so each row's
  scalar is available on all 128 partitions.
"""

import numpy as np

# ---------------------------------------------------------------------------
# problem constants (hardcoded per contract)
B, T, H, V, S = 16, 128, 512, 32000, 400
N_CORES = 8
BPC = B // N_CORES          # batches per core
R_FULL = BPC * T            # rows per core = 256
FD_FULL = V // 128          # 250
SP_FULL = (S + 127) // 128  # 4 s-chunks
G_FULL = 16                 # rows per vocab DMA group

_PROGRAM_CACHE = {}


def build_program(R=R_FULL, FD=FD_FULL, SP=SP_FULL, G=G_FULL, mode="diag",
                  rep=1):
    """Build + compile the per-core Bass program. Same program for all cores.

    R : rows per core (multiple of 128)
    FD: free-dim width of the vocab decomposition (V_local = 128*FD)
    SP: number of 128-wide s-chunks (S padded to SP*128)
    G : rows per vocab/out DMA group
    mode: "diag" -> base p_gen*vocab via a diagonal matmul starting the PSUM
          group; "dve" -> base+merge on VectorE after the scatter matmuls.
    rep : repeat the whole body rep times (identical output; used for
          differential device-time measurement).
    """
    key = (R, FD, SP, G, mode, rep)
    if key in _PROGRAM_CACHE:
        return _PROGRAM_CACHE[key]

    from contextlib import ExitStack

    import concourse.bass as bass
    import concourse.tile as tile
    from concourse import bacc, mybir

    f32 = mybir.dt.float32
    Alu = mybir.AluOpType
    Act = mybir.ActivationFunctionType
    VL = 128 * FD
    RB = R // 128
    NG = R // G
    assert R % 128 == 0 and R % G == 0

    nc = bacc.Bacc("TRN2", target_bir_lowering=False, debug=False)

    ctx_d = nc.dram_tensor("ctx", [R, H], f32, kind="ExternalInput")
    hid_d = nc.dram_tensor("hid", [R, H], f32, kind="ExternalInput")
    trg_d = nc.dram_tensor("trg", [R, H], f32, kind="ExternalInput")
    vocab_d = nc.dram_tensor("vocab", [R, VL], f32, kind="ExternalInput")
    attnT_d = nc.dram_tensor("attnT", [128, RB * SP, 128], f32, kind="ExternalInput")
    piT_d = nc.dram_tensor("piT", [128, RB * SP, 128], f32, kind="ExternalInput")
    fiT_d = nc.dram_tensor("fiT", [128, RB * SP, 128], f32, kind="ExternalInput")
    # weights replicated across partitions on host (pure data movement)
    wh_d = nc.dram_tensor("wh", [128, H], f32, kind="ExternalInput")
    ws_d = nc.dram_tensor("ws", [128, H], f32, kind="ExternalInput")
    wx_d = nc.dram_tensor("wx", [128, H], f32, kind="ExternalInput")
    wxb_d = nc.dram_tensor("wxb", [128, 1], f32, kind="ExternalInput")
    iotaP_d = nc.dram_tensor("iotaP", [128, 128], f32, kind="ExternalInput")
    iotaF_d = nc.dram_tensor("iotaF", [128, FD], f32, kind="ExternalInput")
    ident_d = nc.dram_tensor("ident", [128, 128], f32, kind="ExternalInput")
    out_d = nc.dram_tensor("out", [R, VL], f32, kind="ExternalOutput")

    with tile.TileContext(nc) as tc, ExitStack() as es:
        singles = es.enter_context(tc.tile_pool(name="singles", bufs=1))
        ph1 = es.enter_context(tc.tile_pool(name="ph1", bufs=2))
        vpool = es.enter_context(tc.tile_pool(name="vpool", bufs=2))
        opool = es.enter_context(tc.tile_pool(name="opool", bufs=2))
        abpool = es.enter_context(tc.tile_pool(name="ab", bufs=4))
        ppool = es.enter_context(tc.tile_pool(name="psum", bufs=6, space="PSUM"))
        dpool = es.enter_context(tc.tile_pool(name="dram", bufs=1, space="DRAM"))

        # --- constants / small inputs ---
        attnT = singles.tile([128, RB * SP, 128], f32)
        nc.sync.dma_start(attnT[:], attnT_d[:])
        piT = singles.tile([128, RB * SP, 128], f32)
        nc.sync.dma_start(piT[:], piT_d[:])
        fiT = singles.tile([128, RB * SP, 128], f32)
        nc.sync.dma_start(fiT[:], fiT_d[:])
        iotaP = singles.tile([128, 128], f32)
        nc.sync.dma_start(iotaP[:], iotaP_d[:])
        iotaF = singles.tile([128, FD], f32)
        nc.sync.dma_start(iotaF[:], iotaF_d[:])
        ident = singles.tile([128, 128], f32)
        nc.sync.dma_start(ident[:], ident_d[:])
        wh = singles.tile([128, H], f32)
        nc.sync.dma_start(wh[:], wh_d[:])
        ws = singles.tile([128, H], f32)
        nc.sync.dma_start(ws[:], ws_d[:])
        wx = singles.tile([128, H], f32)
        nc.sync.dma_start(wx[:], wx_d[:])
        wxb = singles.tile([128, 1], f32)
        nc.sync.dma_start(wxb[:], wxb_d[:])
        scaledT = singles.tile([128, RB * SP, 128], f32)
        pgen_all = singles.tile([128, R], f32)
        om_all = singles.tile([128, R], f32)
        pgen_dram = dpool.tile([R, 1], f32)

        # --- phase 1a: p_gen per row (rows on partitions), bounce to DRAM ---
        def _phase1a():
          for blk in range(RB):
            rows = slice(blk * 128, (blk + 1) * 128)
            gacc = ph1.tile([128, 1], f32, tag="gacc")
            gtmp = ph1.tile([128, 1], f32, tag="gtmp")
            g2 = ph1.tile([128, 1], f32, tag="g2")
            prod = ph1.tile([128, H], f32, tag="prod")
            for i, (src_d, w) in enumerate(
                ((ctx_d, wh), (hid_d, ws), (trg_d, wx))
            ):
                x = ph1.tile([128, H], f32, tag="x")
                nc.sync.dma_start(x[:], src_d[rows, :])
                nc.vector.tensor_tensor(prod[:], x[:], w[:], op=Alu.mult)
                dst = (gacc, gtmp, g2)[i]
                nc.vector.tensor_reduce(
                    dst[:], prod[:], axis=mybir.AxisListType.X, op=Alu.add
                )
            gsum = ph1.tile([128, 1], f32, tag="gsum")
            nc.vector.tensor_tensor(gsum[:], gacc[:], gtmp[:], op=Alu.add)
            gall = ph1.tile([128, 1], f32, tag="gall")
            nc.vector.tensor_tensor(gall[:], gsum[:], g2[:], op=Alu.add)
            pgen_col = ph1.tile([128, 1], f32, tag="pgen")
            nc.scalar.activation(
                pgen_col[:], gall[:], Act.Sigmoid, bias=wxb[:], scale=1.0
            )
            nc.sync.dma_start(pgen_dram[rows, :], pgen_col[:])

        # --- phase 1b: broadcast p_gen to all partitions; scaled attnT ---
        def _phase1b():
            pg_flat = pgen_dram[:, 0]
            pg_bcast = bass.AP(
                tensor=pg_flat.tensor, offset=pg_flat.offset,
                ap=[[0, 128]] + list(pg_flat.ap),
            )
            nc.gpsimd.dma_start(pgen_all[:], pg_bcast)
            nc.vector.tensor_scalar(
                om_all[:], pgen_all[:], -1.0, 1.0, Alu.mult, Alu.add
            )
            for blk in range(RB):
                for c in range(SP):
                    nc.vector.tensor_tensor(
                        scaledT[:, blk * SP + c, :],
                        attnT[:, blk * SP + c, :],
                        om_all[:, blk * 128:(blk + 1) * 128],
                        op=Alu.mult,
                    )

        # --- phase 2: per-row scatter-add via one-hot matmuls ---
        vocab_v = vocab_d[:].rearrange("r (p f) -> p r f", p=128)
        out_v = out_d[:].rearrange("r (p f) -> p r f", p=128)

        def _phase2():
          for grp in range(NG):
            gr = slice(grp * G, (grp + 1) * G)
            vt = vpool.tile([128, G, FD], f32)
            nc.sync.dma_start(vt[:], vocab_v[:, gr, :])
            ot = opool.tile([128, G, FD], f32)
            for j in range(G):
                r = grp * G + j
                blk = r // 128
                rl = r % 128
                ps = ppool.tile([128, FD], f32)
                pg_sc = pgen_all[:, r:r + 1]
                if mode == "diag":
                    D = abpool.tile([128, 128], f32, tag="D")
                    nc.vector.tensor_scalar(
                        D[:], ident[:], pg_sc, None, Alu.mult
                    )
                    nc.tensor.matmul(
                        ps[:], lhsT=D[:], rhs=vt[:, j, :],
                        start=True, stop=False,
                    )
                for c in range(SP):
                    ch = blk * SP + c
                    A = abpool.tile([128, 128], f32, tag="A")
                    nc.vector.tensor_scalar(
                        A[:], iotaP[:], piT[:, ch, rl:rl + 1], None, Alu.is_equal
                    )
                    Bt = abpool.tile([128, FD], f32, tag="B")
                    nc.vector.tensor_scalar(
                        Bt[:], iotaF[:], fiT[:, ch, rl:rl + 1],
                        scaledT[:, ch, rl:rl + 1], Alu.is_equal, Alu.mult,
                    )
                    nc.tensor.matmul(
                        ps[:], lhsT=A[:], rhs=Bt[:],
                        start=(False if mode == "diag" else c == 0),
                        stop=(c == SP - 1),
                    )
                if mode == "diag":
                    nc.scalar.copy(ot[:, j, :], ps[:])
                else:
                    nc.vector.tensor_scalar(
                        ot[:, j, :], vt[:, j, :], pg_sc, None, Alu.mult
                    )
                    nc.vector.tensor_tensor(
                        ot[:, j, :], ot[:, j, :], ps[:], op=Alu.add
                    )
            nc.sync.dma_start(out_v[:, gr, :], ot[:])

        for _ in range(rep):
            _phase1a()
            _phase1b()
            _phase2()

    nc.compile()
    _PROGRAM_CACHE[key] = nc
    return nc


def make_core_inputs(ctx, hid, trg, vocab, attn, ids, w_h, w_s, w_x_w, w_x_b,
                     R=R_FULL, FD=FD_FULL, SP=SP_FULL):
    """Host-side prep for one core: flatten rows, decompose + transpose indices.

    ctx/hid/trg: [R, H] f32; vocab: [R, 128*FD] f32; attn: [R, S'] f32;
    ids: [R, S'] int. Returns the in_map dict for this core.
    """
    RB = R // 128
    Sp = SP * 128
    Sl = attn.shape[1]
    f32 = np.float32

    ids = np.asarray(ids).astype(np.int64)
    pi = (ids // FD).astype(f32)
    fi = (ids % FD).astype(f32)

    def tr(x, pad):
        full = np.full((R, Sp), pad, dtype=f32)
        full[:, :Sl] = x
        # [R, Sp] -> [RB, 128(r), SP, 128(s)] -> [s, RB, SP, r]
        t = full.reshape(RB, 128, SP, 128).transpose(3, 0, 2, 1)
        return np.ascontiguousarray(t.reshape(128, RB * SP, 128))

    def rep(w, n):
        return np.ascontiguousarray(
            np.broadcast_to(np.asarray(w, dtype=f32).reshape(1, n), (128, n))
        )

    return {
        "ctx": np.ascontiguousarray(ctx, dtype=f32),
        "hid": np.ascontiguousarray(hid, dtype=f32),
        "trg": np.ascontiguousarray(trg, dtype=f32),
        "vocab": np.ascontiguousarray(vocab, dtype=f32),
        "attnT": tr(np.asarray(attn, dtype=f32), 0.0),
        "piT": tr(pi, 1.0e4),
        "fiT": tr(fi, -1.0),
        "wh": rep(w_h, H),
        "ws": rep(w_s, H),
        "wx": rep(w_x_w, H),
        "wxb": rep(w_x_b, 1),
        "iotaP": rep(np.arange(128, dtype=f32), 128),
        "iotaF": rep(np.arange(FD, dtype=f32), FD),
        "ident": np.eye(128, dtype=f32),
    }


def make_in_maps(context_vecs, hidden, trg_embs, vocab_dists, attn_dists,
                 src_ids, w_h, w_s, w_x_w, w_x_b):
    """Build the 8 per-core input dicts from full inputs."""
    context_vecs = np.asarray(context_vecs)
    hidden = np.asarray(hidden)
    trg_embs = np.asarray(trg_embs)
    vocab_dists = np.asarray(vocab_dists)
    attn_dists = np.asarray(attn_dists)
    src_ids = np.asarray(src_ids)

    in_maps = []
    for i in range(N_CORES):
        bs = slice(i * BPC, (i + 1) * BPC)
        in_maps.append(make_core_inputs(
            context_vecs[bs].reshape(R_FULL, H),
            hidden[bs].reshape(R_FULL, H),
            trg_embs[bs].reshape(R_FULL, H),
            vocab_dists[bs].reshape(R_FULL, V),
            attn_dists[bs].reshape(R_FULL, S),
            src_ids[bs].reshape(R_FULL, S),
            w_h, w_s, w_x_w, w_x_b,
        ))
    return in_maps


def kernel(context_vecs, hidden, trg_embs, vocab_dists, attn_dists,
           src_ids, pad_id, w_h, w_s, w_x_w, w_x_b):
    """Full-input entry point. Shards over 8 NeuronCores, returns [B,T,V] f32."""
    from concourse.bass_utils import run_bass_kernel_spmd

    nc = build_program()
    in_maps = make_in_maps(context_vecs, hidden, trg_embs, vocab_dists,
                           attn_dists, src_ids, w_h, w_s, w_x_w, w_x_b)
    res = run_bass_kernel_spmd(nc, in_maps, list(range(N_CORES)))
    outs = [np.asarray(res.results[i]["out"]).reshape(BPC, T, V)
            for i in range(N_CORES)]
    return np.concatenate(outs, axis=0)


# revision 22
# speedup vs baseline: 19.9845x; 2.4009x over previous
"""CopyMechanism (pointer-generator) kernel for 8 Trainium2 NeuronCores.

Full problem: B=16, T=128, H=512, V=32000, S=400.
  gen = sigmoid(ctx@wh + hid@ws + trg@wx + b)          [B,T,1]
  out = gen * vocab_dists; out[b,t,ids[b,t,s]] += (1-gen)*attn[b,t,s]

Sharding: data-parallel over batch. Core i handles batches [2i, 2i+1]
(256 rows of T-steps). Weights replicated. No cross-core communication.

Device algorithm (per core, per row r):
  Decompose vocab index v = p*250 + f  (V = 128*250), so a row's 32000-wide
  output is an SBUF tile [128 partitions, 250 free].  The scatter-add of the
  S=400 attn values becomes a sum of outer products:
     M[p,f] = sum_s onehot(pi[s])[p] * (onehot(fi[s])[f] * val[s])
  computed by TensorE matmuls contracting s (4 chunks of <=128 on the
  partition axis).  One-hots are built on VectorE with iota/is_equal
  tensor_scalar ops (indices pre-decomposed and pre-transposed on host --
  integer-only preprocessing).  The base p_gen*vocab is pre-written into
  PSUM by ScalarE (activation Copy with per-row scale), matmuls accumulate
  on top (start=False), ScalarE copies PSUM->SBUF, DMA out.

  p_gen is computed on-device (dot products + sigmoid), bounced through a
  DRAM scratch and re-loaded with a partition-broadcast AP so each row's
  scalar is available on all 128 partitions.
"""

import numpy as np
from ml_dtypes import bfloat16

# ---------------------------------------------------------------------------
# problem constants (hardcoded per contract)
B, T, H, V, S = 16, 128, 512, 32000, 400
N_CORES = 8
BPC = B // N_CORES          # batches per core
R_FULL = BPC * T            # rows per core = 256
FD_FULL = V // 128          # 250
SP_FULL = (S + 127) // 128  # 4 s-chunks
G_FULL = 16                 # rows per vocab DMA group

_PROGRAM_CACHE = {}


def build_program(R=R_FULL, FD=FD_FULL, SP=SP_FULL, G=G_FULL, mode="diag",
                  rep=1):
    """Build + compile the per-core Bass program. Same program for all cores.

    R : rows per core (multiple of 128)
    FD: free-dim width of the vocab decomposition (V_local = 128*FD)
    SP: number of 128-wide s-chunks (S padded to SP*128)
    G : rows per vocab/out DMA group
    mode: "diag" -> base p_gen*vocab via a diagonal matmul starting the PSUM
          group; "dve" -> base+merge on VectorE after the scatter matmuls.
    rep : repeat the whole body rep times (identical output; used for
          differential device-time measurement).
    """
    key = (R, FD, SP, G, mode, rep)
    if key in _PROGRAM_CACHE:
        return _PROGRAM_CACHE[key]

    from contextlib import ExitStack

    import concourse.bass as bass
    import concourse.tile as tile
    from concourse import bacc, mybir

    f32 = mybir.dt.float32
    bf16 = mybir.dt.bfloat16
    Alu = mybir.AluOpType
    Act = mybir.ActivationFunctionType
    VL = 128 * FD
    RB = R // 128
    NG = R // G
    assert R % 128 == 0 and R % G == 0

    nc = bacc.Bacc("TRN2", target_bir_lowering=False, debug=False)

    ctx_d = nc.dram_tensor("ctx", [R, H], f32, kind="ExternalInput")
    hid_d = nc.dram_tensor("hid", [R, H], f32, kind="ExternalInput")
    trg_d = nc.dram_tensor("trg", [R, H], f32, kind="ExternalInput")
    vocab_d = nc.dram_tensor("vocab", [R, VL], f32, kind="ExternalInput")
    attnT_d = nc.dram_tensor("attnT", [128, RB * SP, 128], f32, kind="ExternalInput")
    piT_d = nc.dram_tensor("piT", [128, RB * SP, 128], f32, kind="ExternalInput")
    fiT_d = nc.dram_tensor("fiT", [128, RB * SP, 128], f32, kind="ExternalInput")
    # weights replicated across partitions on host (pure data movement)
    wh_d = nc.dram_tensor("wh", [128, H], f32, kind="ExternalInput")
    ws_d = nc.dram_tensor("ws", [128, H], f32, kind="ExternalInput")
    wx_d = nc.dram_tensor("wx", [128, H], f32, kind="ExternalInput")
    wxb_d = nc.dram_tensor("wxb", [128, 1], f32, kind="ExternalInput")
    iotaP_d = nc.dram_tensor("iotaP", [128, 128], bf16, kind="ExternalInput")
    iotaF_d = nc.dram_tensor("iotaF", [128, FD], bf16, kind="ExternalInput")
    ident_d = nc.dram_tensor("ident", [128, 128], f32, kind="ExternalInput")
    out_d = nc.dram_tensor("out", [R, VL], f32, kind="ExternalOutput")

    with tile.TileContext(nc) as tc, ExitStack() as es:
        singles = es.enter_context(tc.tile_pool(name="singles", bufs=1))
        ph1 = es.enter_context(tc.tile_pool(name="ph1", bufs=2))
        vpool = es.enter_context(tc.tile_pool(name="vpool", bufs=2))
        opool = es.enter_context(tc.tile_pool(name="opool", bufs=2))
        abpool = es.enter_context(tc.tile_pool(name="ab", bufs=4))
        ppool = es.enter_context(tc.tile_pool(name="psum", bufs=6, space="PSUM"))
        dpool = es.enter_context(tc.tile_pool(name="dram", bufs=1, space="DRAM"))

        # --- constants / small inputs ---
        attnT = singles.tile([128, RB * SP, 128], f32)
        nc.sync.dma_start(attnT[:], attnT_d[:])
        piT = singles.tile([128, RB * SP, 128], f32)
        nc.sync.dma_start(piT[:], piT_d[:])
        fiT = singles.tile([128, RB * SP, 128], f32)
        nc.sync.dma_start(fiT[:], fiT_d[:])
        iotaP = singles.tile([128, 128], bf16)
        nc.sync.dma_start(iotaP[:], iotaP_d[:])
        iotaF = singles.tile([128, FD], bf16)
        nc.sync.dma_start(iotaF[:], iotaF_d[:])
        ident = singles.tile([128, 128], f32)
        nc.sync.dma_start(ident[:], ident_d[:])
        wh = singles.tile([128, H], f32)
        nc.sync.dma_start(wh[:], wh_d[:])
        ws = singles.tile([128, H], f32)
        nc.sync.dma_start(ws[:], ws_d[:])
        wx = singles.tile([128, H], f32)
        nc.sync.dma_start(wx[:], wx_d[:])
        wxb = singles.tile([128, 1], f32)
        nc.sync.dma_start(wxb[:], wxb_d[:])
        scaledT = singles.tile([128, RB * SP, 128], f32)
        pgen_all = singles.tile([128, R], f32)
        om_all = singles.tile([128, R], f32)
        pgen_dram = dpool.tile([R, 1], f32)

        # --- phase 1a: p_gen per row (rows on partitions), bounce to DRAM ---
        def _phase1a():
          for blk in range(RB):
            rows = slice(blk * 128, (blk + 1) * 128)
            gacc = ph1.tile([128, 1], f32, tag="gacc")
            gtmp = ph1.tile([128, 1], f32, tag="gtmp")
            g2 = ph1.tile([128, 1], f32, tag="g2")
            prod = ph1.tile([128, H], f32, tag="prod")
            for i, (src_d, w) in enumerate(
                ((ctx_d, wh), (hid_d, ws), (trg_d, wx))
            ):
                x = ph1.tile([128, H], f32, tag="x")
                nc.sync.dma_start(x[:], src_d[rows, :])
                nc.vector.tensor_tensor(prod[:], x[:], w[:], op=Alu.mult)
                dst = (gacc, gtmp, g2)[i]
                nc.vector.tensor_reduce(
                    dst[:], prod[:], axis=mybir.AxisListType.X, op=Alu.add
                )
            gsum = ph1.tile([128, 1], f32, tag="gsum")
            nc.vector.tensor_tensor(gsum[:], gacc[:], gtmp[:], op=Alu.add)
            gall = ph1.tile([128, 1], f32, tag="gall")
            nc.vector.tensor_tensor(gall[:], gsum[:], g2[:], op=Alu.add)
            pgen_col = ph1.tile([128, 1], f32, tag="pgen")
            nc.scalar.activation(
                pgen_col[:], gall[:], Act.Sigmoid, bias=wxb[:], scale=1.0
            )
            nc.sync.dma_start(pgen_dram[rows, :], pgen_col[:])

        # --- phase 1b: broadcast p_gen to all partitions; scaled attnT ---
        def _phase1b():
            pg_flat = pgen_dram[:, 0]
            pg_bcast = bass.AP(
                tensor=pg_flat.tensor, offset=pg_flat.offset,
                ap=[[0, 128]] + list(pg_flat.ap),
            )
            nc.gpsimd.dma_start(pgen_all[:], pg_bcast)
            nc.vector.tensor_scalar(
                om_all[:], pgen_all[:], -1.0, 1.0, Alu.mult, Alu.add
            )
            for blk in range(RB):
                for c in range(SP):
                    nc.vector.tensor_tensor(
                        scaledT[:, blk * SP + c, :],
                        attnT[:, blk * SP + c, :],
                        om_all[:, blk * 128:(blk + 1) * 128],
                        op=Alu.mult,
                    )

        # --- phase 2: per-row scatter-add via one-hot matmuls ---
        vocab_v = vocab_d[:].rearrange("r (p f) -> p r f", p=128)
        out_v = out_d[:].rearrange("r (p f) -> p r f", p=128)

        def _phase2():
          for grp in range(NG):
            gr = slice(grp * G, (grp + 1) * G)
            ot = opool.tile([128, G, FD], f32)
            if mode == "dma":
                # Pre-fill ot with per-row p_gen, then the vocab load DMA
                # multiplies in transit: ot = p_gen * vocab (no PE/DVE time).
                for j in range(G):
                    r = grp * G + j
                    nc.scalar.mul(
                        ot[:, j, :],
                        pgen_all[:, r:r + 1].to_broadcast([128, FD]),
                        1.0,
                    )
                nc.gpsimd.dma_start(
                    ot[:], vocab_v[:, gr, :], accum_op=Alu.mult
                )
            else:
                vt = vpool.tile([128, G, FD], f32)
                nc.sync.dma_start(vt[:], vocab_v[:, gr, :])
            for j in range(G):
                r = grp * G + j
                blk = r // 128
                rl = r % 128
                ps = ppool.tile([128, FD], f32)
                pg_sc = pgen_all[:, r:r + 1]
                if mode == "diag":
                    D = abpool.tile([128, 128], f32, tag="D")
                    nc.scalar.mul(D[:], ident[:], pg_sc)
                    nc.tensor.matmul(
                        ps[:], lhsT=D[:], rhs=vt[:, j, :],
                        start=True, stop=False,
                    )
                for c in range(SP):
                    ch = blk * SP + c
                    A = abpool.tile([128, 128], bf16, tag="A")
                    nc.gpsimd.tensor_scalar(
                        A[:], iotaP[:], piT[:, ch, rl:rl + 1], None, Alu.is_equal
                    )
                    Bt = abpool.tile([128, FD], bf16, tag="B")
                    nc.vector.tensor_scalar(
                        Bt[:], iotaF[:], fiT[:, ch, rl:rl + 1],
                        scaledT[:, ch, rl:rl + 1], Alu.is_equal, Alu.mult,
                    )
                    nc.tensor.matmul(
                        ps[:], lhsT=A[:], rhs=Bt[:],
                        start=(False if mode == "diag" else c == 0),
                        stop=(c == SP - 1),
                    )
                if mode == "diag":
                    nc.scalar.copy(ot[:, j, :], ps[:])
                elif mode == "dma":
                    nc.vector.tensor_tensor(
                        ot[:, j, :], ot[:, j, :], ps[:], op=Alu.add
                    )
                else:
                    nc.vector.tensor_scalar(
                        ot[:, j, :], vt[:, j, :], pg_sc, None, Alu.mult
                    )
                    nc.vector.tensor_tensor(
                        ot[:, j, :], ot[:, j, :], ps[:], op=Alu.add
                    )
            nc.sync.dma_start(out_v[:, gr, :], ot[:])

        for _ in range(rep):
            _phase1a()
            _phase1b()
            _phase2()

    nc.compile()
    _PROGRAM_CACHE[key] = nc
    return nc


def make_core_inputs(ctx, hid, trg, vocab, attn, ids, w_h, w_s, w_x_w, w_x_b,
                     R=R_FULL, FD=FD_FULL, SP=SP_FULL):
    """Host-side prep for one core: flatten rows, decompose + transpose indices.

    ctx/hid/trg: [R, H] f32; vocab: [R, 128*FD] f32; attn: [R, S'] f32;
    ids: [R, S'] int. Returns the in_map dict for this core.
    """
    RB = R // 128
    Sp = SP * 128
    Sl = attn.shape[1]
    f32 = np.float32

    ids = np.asarray(ids).astype(np.int64)
    pi = (ids // FD).astype(f32)
    fi = (ids % FD).astype(f32)

    def tr(x, pad):
        full = np.full((R, Sp), pad, dtype=f32)
        full[:, :Sl] = x
        # [R, Sp] -> [RB, 128(r), SP, 128(s)] -> [s, RB, SP, r]
        t = full.reshape(RB, 128, SP, 128).transpose(3, 0, 2, 1)
        return np.ascontiguousarray(t.reshape(128, RB * SP, 128))

    def rep(w, n):
        return np.ascontiguousarray(
            np.broadcast_to(np.asarray(w, dtype=f32).reshape(1, n), (128, n))
        )

    return {
        "ctx": np.ascontiguousarray(ctx, dtype=f32),
        "hid": np.ascontiguousarray(hid, dtype=f32),
        "trg": np.ascontiguousarray(trg, dtype=f32),
        "vocab": np.ascontiguousarray(vocab, dtype=f32),
        "attnT": tr(np.asarray(attn, dtype=f32), 0.0),
        "piT": tr(pi, 1.0e4),
        "fiT": tr(fi, -1.0),
        "wh": rep(w_h, H),
        "ws": rep(w_s, H),
        "wx": rep(w_x_w, H),
        "wxb": rep(w_x_b, 1),
        "iotaP": rep(np.arange(128, dtype=f32), 128).astype(bfloat16),
        "iotaF": rep(np.arange(FD, dtype=f32), FD).astype(bfloat16),
        "ident": np.eye(128, dtype=f32),
    }


def make_in_maps(context_vecs, hidden, trg_embs, vocab_dists, attn_dists,
                 src_ids, w_h, w_s, w_x_w, w_x_b):
    """Build the 8 per-core input dicts from full inputs."""
    context_vecs = np.asarray(context_vecs)
    hidden = np.asarray(hidden)
    trg_embs = np.asarray(trg_embs)
    vocab_dists = np.asarray(vocab_dists)
    attn_dists = np.asarray(attn_dists)
    src_ids = np.asarray(src_ids)

    in_maps = []
    for i in range(N_CORES):
        bs = slice(i * BPC, (i + 1) * BPC)
        in_maps.append(make_core_inputs(
            context_vecs[bs].reshape(R_FULL, H),
            hidden[bs].reshape(R_FULL, H),
            trg_embs[bs].reshape(R_FULL, H),
            vocab_dists[bs].reshape(R_FULL, V),
            attn_dists[bs].reshape(R_FULL, S),
            src_ids[bs].reshape(R_FULL, S),
            w_h, w_s, w_x_w, w_x_b,
        ))
    return in_maps


def kernel(context_vecs, hidden, trg_embs, vocab_dists, attn_dists,
           src_ids, pad_id, w_h, w_s, w_x_w, w_x_b):
    """Full-input entry point. Shards over 8 NeuronCores, returns [B,T,V] f32."""
    from concourse.bass_utils import run_bass_kernel_spmd

    nc = build_program()
    in_maps = make_in_maps(context_vecs, hidden, trg_embs, vocab_dists,
                           attn_dists, src_ids, w_h, w_s, w_x_w, w_x_b)
    res = run_bass_kernel_spmd(nc, in_maps, list(range(N_CORES)))
    outs = [np.asarray(res.results[i]["out"]).reshape(BPC, T, V)
            for i in range(N_CORES)]
    return np.concatenate(outs, axis=0)


# revision 30
# speedup vs baseline: 233.8755x; 11.7028x over previous
"""CopyMechanism (pointer-generator) kernel for 8 Trainium2 NeuronCores.

Full problem: B=16, T=128, H=512, V=32000, S=400.
  gen = sigmoid(ctx@wh + hid@ws + trg@wx + b)          [B,T,1]
  out = gen * vocab_dists; out[b,t,ids[b,t,s]] += (1-gen)*attn[b,t,s]

Sharding: data-parallel over batch. Core i handles batches [2i, 2i+1]
(256 rows of T-steps). Weights replicated. No cross-core communication.

Device algorithm (per core, per row r):
  Decompose vocab index v = p*250 + f  (V = 128*250), so a row's 32000-wide
  output is an SBUF tile [128 partitions, 250 free].  The scatter-add of the
  S=400 attn values becomes a sum of outer products:
     M[p,f] = sum_s onehot(pi[s])[p] * (onehot(fi[s])[f] * val[s])
  computed by TensorE matmuls contracting s (4 chunks of <=128 on the
  partition axis).  One-hots are built on VectorE with iota/is_equal
  tensor_scalar ops (indices pre-decomposed and pre-transposed on host --
  integer-only preprocessing).  The base p_gen*vocab is pre-written into
  PSUM by ScalarE (activation Copy with per-row scale), matmuls accumulate
  on top (start=False), ScalarE copies PSUM->SBUF, DMA out.

  p_gen is computed on-device (dot products + sigmoid), bounced through a
  DRAM scratch and re-loaded with a partition-broadcast AP so each row's
  scalar is available on all 128 partitions.
"""

import numpy as np
from ml_dtypes import bfloat16

# ---------------------------------------------------------------------------
# problem constants (hardcoded per contract)
B, T, H, V, S = 16, 128, 512, 32000, 400
N_CORES = 8
BPC = B // N_CORES          # batches per core
R_FULL = BPC * T            # rows per core = 256
FD_FULL = V // 128          # 250
SP_FULL = (S + 127) // 128  # 4 s-chunks
G_FULL = 16                 # rows per vocab DMA group

_PROGRAM_CACHE = {}


def build_program(R=R_FULL, FD=FD_FULL, SP=SP_FULL, G=G_FULL, mode="diag",
                  rep=1, a_engine="dve", ablate="full"):
    """Build + compile the per-core Bass program. Same program for all cores.

    R : rows per core (multiple of 128)
    FD: free-dim width of the vocab decomposition (V_local = 128*FD)
    SP: number of 128-wide s-chunks (S padded to SP*128)
    G : rows per vocab/out DMA group
    mode: "diag" -> base p_gen*vocab via a diagonal matmul starting the PSUM
          group; "dve" -> base+merge on VectorE after the scatter matmuls.
    rep : repeat the whole body rep times (identical output; used for
          differential device-time measurement).
    """
    key = (R, FD, SP, G, mode, rep, a_engine, ablate)
    if key in _PROGRAM_CACHE:
        return _PROGRAM_CACHE[key]

    from contextlib import ExitStack

    import concourse.bass as bass
    import concourse.tile as tile
    from concourse import bacc, mybir

    f32 = mybir.dt.float32
    bf16 = mybir.dt.bfloat16
    Alu = mybir.AluOpType
    Act = mybir.ActivationFunctionType
    VL = 128 * FD
    RB = R // 128
    NG = R // G
    assert R % 128 == 0 and R % G == 0

    nc = bacc.Bacc("TRN2", target_bir_lowering=False, debug=False)

    ctx_d = nc.dram_tensor("ctx", [R, H], f32, kind="ExternalInput")
    hid_d = nc.dram_tensor("hid", [R, H], f32, kind="ExternalInput")
    trg_d = nc.dram_tensor("trg", [R, H], f32, kind="ExternalInput")
    vocab_d = nc.dram_tensor("vocab", [R, VL], f32, kind="ExternalInput")
    attnT_d = nc.dram_tensor("attnT", [128, RB * SP, 128], f32, kind="ExternalInput")
    piT_d = nc.dram_tensor("piT", [128, RB * SP, 128], f32, kind="ExternalInput")
    fiT_d = nc.dram_tensor("fiT", [128, RB * SP, 128], f32, kind="ExternalInput")
    # weights replicated across partitions on host (pure data movement)
    wh_d = nc.dram_tensor("wh", [128, H], f32, kind="ExternalInput")
    ws_d = nc.dram_tensor("ws", [128, H], f32, kind="ExternalInput")
    wx_d = nc.dram_tensor("wx", [128, H], f32, kind="ExternalInput")
    wxb_d = nc.dram_tensor("wxb", [128, 1], f32, kind="ExternalInput")
    iotaP_d = nc.dram_tensor("iotaP", [128, 128], bf16, kind="ExternalInput")
    iotaF_d = nc.dram_tensor("iotaF", [128, FD], bf16, kind="ExternalInput")
    ident_d = nc.dram_tensor("ident", [128, 128], f32, kind="ExternalInput")
    out_d = nc.dram_tensor("out", [R, VL], f32, kind="ExternalOutput")

    with tile.TileContext(nc) as tc, ExitStack() as es:
        singles = es.enter_context(tc.tile_pool(name="singles", bufs=1))
        ph1 = es.enter_context(tc.tile_pool(name="ph1", bufs=2))
        vpool = es.enter_context(tc.tile_pool(name="vpool", bufs=3))
        opool = es.enter_context(tc.tile_pool(name="opool", bufs=3))
        abpool = es.enter_context(tc.tile_pool(name="ab", bufs=6))
        ppool = es.enter_context(tc.tile_pool(name="psum", bufs=8, space="PSUM"))
        dpool = es.enter_context(tc.tile_pool(name="dram", bufs=1, space="DRAM"))

        # --- constants / small inputs ---
        attnT = singles.tile([128, RB * SP, 128], f32)
        nc.sync.dma_start(attnT[:], attnT_d[:])
        piT = singles.tile([128, RB * SP, 128], f32)
        nc.sync.dma_start(piT[:], piT_d[:])
        fiT = singles.tile([128, RB * SP, 128], f32)
        nc.sync.dma_start(fiT[:], fiT_d[:])
        iotaP = singles.tile([128, 128], bf16)
        nc.sync.dma_start(iotaP[:], iotaP_d[:])
        iotaF = singles.tile([128, FD], bf16)
        nc.sync.dma_start(iotaF[:], iotaF_d[:])
        ident = singles.tile([128, 128], f32)
        nc.sync.dma_start(ident[:], ident_d[:])
        wh = singles.tile([128, H], f32)
        nc.sync.dma_start(wh[:], wh_d[:])
        ws = singles.tile([128, H], f32)
        nc.sync.dma_start(ws[:], ws_d[:])
        wx = singles.tile([128, H], f32)
        nc.sync.dma_start(wx[:], wx_d[:])
        wxb = singles.tile([128, 1], f32)
        nc.sync.dma_start(wxb[:], wxb_d[:])
        scaledT = singles.tile([128, RB * SP, 128], f32)
        pgen_all = singles.tile([128, R], f32)
        om_all = singles.tile([128, R], f32)
        pgen_dram = dpool.tile([R, 1], f32)

        # --- phase 1a: p_gen per row (rows on partitions), bounce to DRAM ---
        def _phase1a():
          for blk in range(RB):
            rows = slice(blk * 128, (blk + 1) * 128)
            gacc = ph1.tile([128, 1], f32, tag="gacc")
            gtmp = ph1.tile([128, 1], f32, tag="gtmp")
            g2 = ph1.tile([128, 1], f32, tag="g2")
            prod = ph1.tile([128, H], f32, tag="prod")
            for i, (src_d, w) in enumerate(
                ((ctx_d, wh), (hid_d, ws), (trg_d, wx))
            ):
                x = ph1.tile([128, H], f32, tag="x")
                nc.sync.dma_start(x[:], src_d[rows, :])
                nc.vector.tensor_tensor(prod[:], x[:], w[:], op=Alu.mult)
                dst = (gacc, gtmp, g2)[i]
                nc.vector.tensor_reduce(
                    dst[:], prod[:], axis=mybir.AxisListType.X, op=Alu.add
                )
            gsum = ph1.tile([128, 1], f32, tag="gsum")
            nc.vector.tensor_tensor(gsum[:], gacc[:], gtmp[:], op=Alu.add)
            gall = ph1.tile([128, 1], f32, tag="gall")
            nc.vector.tensor_tensor(gall[:], gsum[:], g2[:], op=Alu.add)
            pgen_col = ph1.tile([128, 1], f32, tag="pgen")
            nc.scalar.activation(
                pgen_col[:], gall[:], Act.Sigmoid, bias=wxb[:], scale=1.0
            )
            nc.sync.dma_start(pgen_dram[rows, :], pgen_col[:])

        # --- phase 1b: broadcast p_gen to all partitions; scaled attnT ---
        def _phase1b():
            pg_flat = pgen_dram[:, 0]
            pg_bcast = bass.AP(
                tensor=pg_flat.tensor, offset=pg_flat.offset,
                ap=[[0, 128]] + list(pg_flat.ap),
            )
            nc.gpsimd.dma_start(pgen_all[:], pg_bcast)
            nc.vector.tensor_scalar(
                om_all[:], pgen_all[:], -1.0, 1.0, Alu.mult, Alu.add
            )
            for blk in range(RB):
                for c in range(SP):
                    nc.vector.tensor_tensor(
                        scaledT[:, blk * SP + c, :],
                        attnT[:, blk * SP + c, :],
                        om_all[:, blk * 128:(blk + 1) * 128],
                        op=Alu.mult,
                    )

        # --- phase 2: per-row scatter-add via one-hot matmuls ---
        vocab_v = vocab_d[:].rearrange("r (p f) -> p r f", p=128)
        out_v = out_d[:].rearrange("r (p f) -> p r f", p=128)

        def _phase2():
          for grp in range(NG):
            gr = slice(grp * G, (grp + 1) * G)
            ot = opool.tile([128, G, FD], f32)
            if mode == "dma":
                # Pre-fill ot with per-row p_gen, then the vocab load DMA
                # multiplies in transit: ot = p_gen * vocab (no PE/DVE time).
                for j in range(G):
                    r = grp * G + j
                    # ot[:, j, :] = 0*iotaF + p_gen[r]  (no broadcast APs)
                    nc.scalar.activation(
                        ot[:, j, :], iotaF[:], Act.Identity,
                        bias=pgen_all[:, r:r + 1], scale=0.0,
                    )
                nc.gpsimd.dma_start(
                    ot[:], vocab_v[:, gr, :], accum_op=Alu.mult
                )
            else:
                vt = vpool.tile([128, G, FD], f32)
                nc.sync.dma_start(vt[:], vocab_v[:, gr, :])
            if ablate == "dmaonly":
                if mode != "dma":
                    nc.scalar.copy(ot[:, :, :], vt[:, :, :])
                nc.scalar.dma_start(out_v[:, gr, :], ot[:])
                continue
            for j in range(G):
                r = grp * G + j
                blk = r // 128
                rl = r % 128
                if ablate == "nomm":
                    nc.scalar.copy(ot[:, j, :], vt[:, j, :])
                ps = ppool.tile([128, FD], f32)
                pg_sc = pgen_all[:, r:r + 1]
                if ablate == "nomm":
                    for c in range(SP):
                        ch = blk * SP + c
                        A = abpool.tile([128, 128], bf16, tag="A")
                        eng = nc.gpsimd if a_engine == "gpsimd" else nc.vector
                        eng.tensor_scalar(
                            A[:], iotaP[:], piT[:, ch, rl:rl + 1],
                            scaledT[:, ch, rl:rl + 1], Alu.is_equal, Alu.mult,
                        )
                        Bt = abpool.tile([128, FD], bf16, tag="B")
                        nc.vector.tensor_scalar(
                            Bt[:], iotaF[:], fiT[:, ch, rl:rl + 1], None,
                            Alu.is_equal,
                        )
                    continue
                if mode == "diag":
                    D = abpool.tile([128, 128], f32, tag="D")
                    nc.scalar.mul(D[:], ident[:], pg_sc)
                    nc.tensor.matmul(
                        ps[:], lhsT=D[:], rhs=vt[:, j, :],
                        start=True, stop=False,
                    )
                for c in range(SP):
                    ch = blk * SP + c
                    # A carries the value: A[s,p] = (pi[s]==p) * val[s]
                    A = abpool.tile([128, 128], bf16, tag="A")
                    a_eng = nc.gpsimd if a_engine == "gpsimd" else nc.vector
                    a_eng.tensor_scalar(
                        A[:], iotaP[:], piT[:, ch, rl:rl + 1],
                        scaledT[:, ch, rl:rl + 1], Alu.is_equal, Alu.mult,
                    )
                    # B is the pure one-hot of fi (1-op, wide)
                    Bt = abpool.tile([128, FD], bf16, tag="B")
                    nc.vector.tensor_scalar(
                        Bt[:], iotaF[:], fiT[:, ch, rl:rl + 1], None,
                        Alu.is_equal,
                    )
                    nc.tensor.matmul(
                        ps[:], lhsT=A[:], rhs=Bt[:],
                        start=(False if mode == "diag" else c == 0),
                        stop=(c == SP - 1),
                    )
                if mode == "diag":
                    nc.scalar.copy(ot[:, j, :], ps[:])
                elif mode == "dma":
                    nc.vector.tensor_tensor(
                        ot[:, j, :], ot[:, j, :], ps[:], op=Alu.add
                    )
                else:
                    nc.vector.tensor_scalar(
                        ot[:, j, :], vt[:, j, :], pg_sc, None, Alu.mult
                    )
                    nc.vector.tensor_tensor(
                        ot[:, j, :], ot[:, j, :], ps[:], op=Alu.add
                    )
            nc.scalar.dma_start(out_v[:, gr, :], ot[:])

        for _ in range(rep):
            _phase1a()
            _phase1b()
            _phase2()

    nc.compile()
    _PROGRAM_CACHE[key] = nc
    return nc


def make_core_inputs(ctx, hid, trg, vocab, attn, ids, w_h, w_s, w_x_w, w_x_b,
                     R=R_FULL, FD=FD_FULL, SP=SP_FULL):
    """Host-side prep for one core: flatten rows, decompose + transpose indices.

    ctx/hid/trg: [R, H] f32; vocab: [R, 128*FD] f32; attn: [R, S'] f32;
    ids: [R, S'] int. Returns the in_map dict for this core.
    """
    RB = R // 128
    Sp = SP * 128
    Sl = attn.shape[1]
    f32 = np.float32

    ids = np.asarray(ids).astype(np.int64)
    pi = (ids // FD).astype(f32)
    fi = (ids % FD).astype(f32)

    def tr(x, pad):
        full = np.full((R, Sp), pad, dtype=f32)
        full[:, :Sl] = x
        # [R, Sp] -> [RB, 128(r), SP, 128(s)] -> [s, RB, SP, r]
        t = full.reshape(RB, 128, SP, 128).transpose(3, 0, 2, 1)
        return np.ascontiguousarray(t.reshape(128, RB * SP, 128))

    def rep(w, n):
        return np.ascontiguousarray(
            np.broadcast_to(np.asarray(w, dtype=f32).reshape(1, n), (128, n))
        )

    return {
        "ctx": np.ascontiguousarray(ctx, dtype=f32),
        "hid": np.ascontiguousarray(hid, dtype=f32),
        "trg": np.ascontiguousarray(trg, dtype=f32),
        "vocab": np.ascontiguousarray(vocab, dtype=f32),
        "attnT": tr(np.asarray(attn, dtype=f32), 0.0),
        "piT": tr(pi, 1.0e4),
        "fiT": tr(fi, -1.0),
        "wh": rep(w_h, H),
        "ws": rep(w_s, H),
        "wx": rep(w_x_w, H),
        "wxb": rep(w_x_b, 1),
        "iotaP": rep(np.arange(128, dtype=f32), 128).astype(bfloat16),
        "iotaF": rep(np.arange(FD, dtype=f32), FD).astype(bfloat16),
        "ident": np.eye(128, dtype=f32),
    }


def make_in_maps(context_vecs, hidden, trg_embs, vocab_dists, attn_dists,
                 src_ids, w_h, w_s, w_x_w, w_x_b):
    """Build the 8 per-core input dicts from full inputs."""
    context_vecs = np.asarray(context_vecs)
    hidden = np.asarray(hidden)
    trg_embs = np.asarray(trg_embs)
    vocab_dists = np.asarray(vocab_dists)
    attn_dists = np.asarray(attn_dists)
    src_ids = np.asarray(src_ids)

    in_maps = []
    for i in range(N_CORES):
        bs = slice(i * BPC, (i + 1) * BPC)
        in_maps.append(make_core_inputs(
            context_vecs[bs].reshape(R_FULL, H),
            hidden[bs].reshape(R_FULL, H),
            trg_embs[bs].reshape(R_FULL, H),
            vocab_dists[bs].reshape(R_FULL, V),
            attn_dists[bs].reshape(R_FULL, S),
            src_ids[bs].reshape(R_FULL, S),
            w_h, w_s, w_x_w, w_x_b,
        ))
    return in_maps


def kernel(context_vecs, hidden, trg_embs, vocab_dists, attn_dists,
           src_ids, pad_id, w_h, w_s, w_x_w, w_x_b):
    """Full-input entry point. Shards over 8 NeuronCores, returns [B,T,V] f32."""
    from concourse.bass_utils import run_bass_kernel_spmd

    nc = build_program()
    in_maps = make_in_maps(context_vecs, hidden, trg_embs, vocab_dists,
                           attn_dists, src_ids, w_h, w_s, w_x_w, w_x_b)
    res = run_bass_kernel_spmd(nc, in_maps, list(range(N_CORES)))
    outs = [np.asarray(res.results[i]["out"]).reshape(BPC, T, V)
            for i in range(N_CORES)]
    return np.concatenate(outs, axis=0)
